# revision 1
# baseline (speedup 1.0000x reference)
"""GCN 2-layer kernel for trn2: host preprocessing + Bass kernel builder.

Math (per GCNConv, PyG-style):
  out = D^-1/2 (A+I) D^-1/2 (X W) + b
Layer1 -> relu -> Layer2.

Device plan (8 cores, SPMD):
  P1: h1' = dinv .* (x_shard @ W1)          (node-sharded, x^T fed from host)
  AG1: allgather h1' -> [NP, HID]
  P3: per 128-dst block: gather h1'[src] (dma_gather int16, 4 quadrant calls),
      indicator matmul -> psum [HID, 128d] (transposed), dinv[d] scale (DVE),
      relu(+b1) (ACT) -> relu1T, @W2 -> psum [128n, CPAD], dinv scale -> h2'
  AG2: allgather h2' -> [NP, CPAD]  (pi-order rows)
  P5: per block: gather h2'[src-pi], indicator matmul -> psum [128d, CPAD],
      dinv scale + b2 -> out rows (pi-order)
Host: unpermute rows, slice [:N0, :CLS].
"""

from dataclasses import dataclass, field

import numpy as np

import concourse.bass as bass
import concourse.mybir as mybir
import concourse.tile as tile
from concourse import bacc

FP = mybir.dt.float32


@dataclass
class Cfg:
    N0: int = 100000     # real nodes
    W: int = 8           # cores
    SHARD: int = 12544   # nodes per core (multiple of 128)
    F: int = 512         # in features (multiple of 128)
    HID: int = 128
    CLS: int = 40
    CPAD: int = 128
    Q: int = 4           # quadrants for int16 gather indexing
    BF16: bool = True    # bf16 aggregation path

    @property
    def NP(self):
        return self.W * self.SHARD

    @property
    def QS(self):
        return self.NP // self.Q

    @property
    def NB(self):
        return self.SHARD // 128  # blocks per core


@dataclass
class Meta:
    # per (block) chunk counts per quadrant, shared across cores: [NB][Q]
    kq1: np.ndarray = None  # int [NB, Q]
    kq2: np.ndarray = None
    node_of_pos: np.ndarray = None  # [W, SHARD] -> node id (or -1 pad)
    cpb1: np.ndarray = None  # [NB] = kq1.sum(1)
    cpb2: np.ndarray = None


def _route_edges(cfg, src, dst_pos, srckey, NBW):
    """Bucket edges by (coreblock = pos-block of dst, quadrant of srckey).
    Returns per-core padded chunk-stream arrays + kq table.

    src: value gathered (index into gather table, 0..NP)
    dst_pos: position-order id of dst (c*SHARD + bb*128 + slot)
    srckey: same as src (quadrant derived from it)
    """
    W, SHARD, Q, QS, NB = cfg.W, cfg.SHARD, cfg.Q, cfg.QS, cfg.NB
    c = dst_pos // SHARD
    bb = (dst_pos % SHARD) // 128
    slot = dst_pos % 128
    q = srckey // QS
    lidx = srckey % QS

    # sort edges by (c, bb, q)
    key = (c.astype(np.int64) * NB + bb) * Q + q
    order = np.argsort(key, kind="stable")
    key_s = key[order]
    lidx_s = lidx[order].astype(np.int64)
    slot_s = slot[order].astype(np.int64)

    nseg = W * NB * Q
    counts = np.bincount(key_s, minlength=nseg).reshape(W, NB, Q)
    kq = np.maximum(np.ceil(counts.max(axis=0) / 128).astype(np.int64), 1)  # [NB, Q]
    cpb = kq.sum(axis=1)  # [NB]
    CT = int(cpb.sum())

    # destination offsets for each (c, bb, q) segment inside the padded stream
    # padded segment length (in edges) = kq[bb, q]*128, same for all cores
    seg_pad = (kq * 128)  # [NB, Q]
    # offset of segment (bb, q) in the per-core padded edge stream:
    seg_off = np.zeros((NB, Q), dtype=np.int64)
    flat = seg_pad.reshape(-1)
    seg_off.reshape(-1)[1:] = np.cumsum(flat)[:-1]
    EPAD = int(seg_pad.sum())
    assert EPAD == CT * 128

    # build padded arrays: idx (int64 for now), dstloc fp32 (-1 pad)
    idx_pad = np.zeros((W, EPAD), dtype=np.int64)
    dl_pad = np.full((W, EPAD), -1.0, dtype=np.float32)

    seg_start = np.zeros(nseg + 1, dtype=np.int64)
    seg_start[1:] = np.cumsum(counts.reshape(-1))
    for ci in range(W):
        for bbi in range(NB):
            for qi in range(Q):
                sidx = (ci * NB + bbi) * Q + qi
                s0, s1 = seg_start[sidx], seg_start[sidx + 1]
                n = s1 - s0
                if n == 0:
                    continue
                o = seg_off[bbi, qi]
                idx_pad[ci, o:o + n] = lidx_s[s0:s1]
                dl_pad[ci, o:o + n] = slot_s[s0:s1]

    # wrapped int16 idx layout [128, CT*8] and dstloc [128, CT]
    idx16 = np.zeros((W, 128, CT * 8), dtype=np.int16)
    dstloc = np.zeros((W, 128, CT), dtype=np.float32)
    for ci in range(W):
        a = idx_pad[ci].reshape(CT, 8, 16)  # chunk, word, lane
        wrapped = a.transpose(2, 0, 1).reshape(16, CT * 8)
        idx16[ci] = np.tile(wrapped, (8, 1)).astype(np.int16)
        dstloc[ci] = dl_pad[ci].reshape(CT, 128).T
    return idx16, dstloc, kq, cpb


def preprocess(cfg: Cfg, x, edge_index, W1, b1, W2, b2):
    N0, W, SHARD, NP = cfg.N0, cfg.W, cfg.SHARD, cfg.NP
    NB, Q = cfg.NB, cfg.Q
    x = np.asarray(x, dtype=np.float32)
    edge_index = np.asarray(edge_index)
    W1 = np.asarray(W1, np.float32)
    b1 = np.asarray(b1, np.float32)
    W2 = np.asarray(W2, np.float32)
    b2 = np.asarray(b2, np.float32)

    s = edge_index[0].astype(np.int64)
    d = edge_index[1].astype(np.int64)
    loops = np.arange(N0, dtype=np.int64)
    s_all = np.concatenate([s, loops])
    d_all = np.concatenate([d, loops])

    deg = np.bincount(d_all, minlength=NP).astype(np.float64)
    dinv = np.where(deg > 0, 1.0 / np.sqrt(deg), 0.0).astype(np.float32)  # [NP]

    # --- degree-balanced block assignment (serpentine over all W*NB blocks) ---
    nblocks = W * NB
    order = np.argsort(-deg[:N0], kind="stable")  # real nodes by deg desc
    # pad ids fill the tail slots
    all_ids = np.concatenate([order, np.arange(N0, NP, dtype=np.int64)])
    r = np.arange(NP, dtype=np.int64)
    cyc = r % (2 * nblocks)
    blk = np.where(cyc < nblocks, cyc, 2 * nblocks - 1 - cyc)  # serpentine block id
    slot_ctr = r // (2 * nblocks) * 2 + (cyc >= nblocks).astype(np.int64)
    # global block gb -> (core, bb): core = gb % W, bb = gb // W
    pos = (gb_core := blk % W) * SHARD + (blk // W) * 128 + slot_ctr
    pos_of_node = np.empty(NP, dtype=np.int64)
    pos_of_node[all_ids] = pos
    node_of_pos = np.empty(NP, dtype=np.int64)
    node_of_pos[pos] = all_ids

    # --- layer-1 edge tables: gather index = original src id ---
    dst_pos = pos_of_node[d_all]
    idx16_1, dstloc1, kq1, cpb1 = _route_edges(cfg, s_all, dst_pos, s_all, NB)
    # --- layer-2 edge tables: gather index = pos id of src; self-loops
    # excluded (added diagonally on-device from local ag2_in rows) ---
    E0 = len(s)
    src_pos = pos_of_node[s_all[:E0]]
    idx16_2, dstloc2, kq2, cpb2 = _route_edges(cfg, src_pos, dst_pos[:E0],
                                               src_pos, NB)

    dinv_pos = dinv[node_of_pos.reshape(W, SHARD)]  # [W, SHARD] pi-order

    per_core = []
    for c in range(W):
        xs = np.zeros((SHARD, cfg.F), np.float32)
        lo, hi = c * SHARD, min((c + 1) * SHARD, N0)
        if hi > lo:
            xs[: hi - lo] = x[lo:hi]
        import ml_dtypes
        bft = ml_dtypes.bfloat16 if cfg.BF16 else np.float32
        inp = {
            "xT": np.ascontiguousarray(xs.T).astype(bft),          # [F, SHARD]
            "w1": W1.astype(bft),                                  # [F, HID]
            "b1col": b1.reshape(cfg.HID, 1).copy(),                # [HID, 1]
            "w2p": np.pad(W2, ((0, 0), (0, cfg.CPAD - cfg.CLS))).astype(bft),
            "b2rep": np.broadcast_to(
                np.pad(b2, (0, cfg.CPAD - cfg.CLS)), (128, cfg.CPAD)).copy(),
            "iota": np.broadcast_to(
                np.arange(128, dtype=np.float32), (128, 128)).copy(),
            "idx1": idx16_1[c], "dl1": dstloc1[c],
            "idx2": idx16_2[c], "dl2": dstloc2[c],
            "dinv_x": dinv[c * SHARD:(c + 1) * SHARD].reshape(SHARD, 1).copy(),
            "dinv_pc": dinv_pos[c].reshape(SHARD, 1).copy(),
            "dinv2_pc": (dinv_pos[c].astype(np.float64) ** 2
                         ).astype(np.float32).reshape(SHARD, 1),
            "dinv_pr": np.broadcast_to(dinv_pos[c], (128, SHARD)).copy(),
        }
        per_core.append(inp)

    meta = Meta(kq1=kq1, kq2=kq2, node_of_pos=node_of_pos.reshape(W, SHARD),
                cpb1=cpb1, cpb2=cpb2)
    return per_core, meta, dinv


def postprocess(cfg: Cfg, outs, meta: Meta):
    """outs: list of [SHARD, CPAD] per core -> [N0, CLS] in node order."""
    res = np.zeros((cfg.NP, cfg.CPAD), np.float32)
    for c in range(cfg.W):
        res[meta.node_of_pos[c]] = outs[c]
    return res[:cfg.N0, :cfg.CLS]


def build(cfg: Cfg, meta: Meta):
    W, SHARD, NP, F, HID, CPAD = cfg.W, cfg.SHARD, cfg.NP, cfg.F, cfg.HID, cfg.CPAD
    NB, Q, QS = cfg.NB, cfg.Q, cfg.QS
    kq1, kq2 = meta.kq1, meta.kq2
    cpb1, cpb2 = meta.cpb1, meta.cpb2
    CT1, CT2 = int(cpb1.sum()), int(cpb2.sum())
    MAXCPB1, MAXCPB2 = int(cpb1.max()), int(cpb2.max())
    KT = F // 128

    nc = bacc.Bacc("TRN2", target_bir_lowering=False, debug=False,
                   num_devices=W, num_swdge_queues=4)

    BFIN = mybir.dt.bfloat16 if cfg.BF16 else FP
    xT = nc.dram_tensor("xT", [F, SHARD], BFIN, kind="ExternalInput")
    w1 = nc.dram_tensor("w1", [F, HID], BFIN, kind="ExternalInput")
    b1col = nc.dram_tensor("b1col", [HID, 1], FP, kind="ExternalInput")
    w2p = nc.dram_tensor("w2p", [HID, CPAD], BFIN, kind="ExternalInput")
    b2rep = nc.dram_tensor("b2rep", [128, CPAD], FP, kind="ExternalInput")
    iota = nc.dram_tensor("iota", [128, 128], FP, kind="ExternalInput")
    idx1 = nc.dram_tensor("idx1", [128, CT1 * 8], mybir.dt.int16, kind="ExternalInput")
    dl1 = nc.dram_tensor("dl1", [128, CT1], FP, kind="ExternalInput")
    idx2 = nc.dram_tensor("idx2", [128, CT2 * 8], mybir.dt.int16, kind="ExternalInput")
    dl2 = nc.dram_tensor("dl2", [128, CT2], FP, kind="ExternalInput")
    dinv_x = nc.dram_tensor("dinv_x", [SHARD, 1], FP, kind="ExternalInput")
    dinv_pc = nc.dram_tensor("dinv_pc", [SHARD, 1], FP, kind="ExternalInput")
    dinv2_pc = nc.dram_tensor("dinv2_pc", [SHARD, 1], FP, kind="ExternalInput")
    dinv_pr = nc.dram_tensor("dinv_pr", [128, SHARD], FP, kind="ExternalInput")
    out_s = nc.dram_tensor("out_s", [SHARD, CPAD], FP, kind="ExternalOutput")

    BF = mybir.dt.bfloat16 if cfg.BF16 else FP
    ag1_in = nc.dram_tensor("ag1_in", [SHARD, HID], BF)
    ag1_out = nc.dram_tensor("ag1_out", [NP, HID], BF, addr_space="Shared")
    ag2_in = nc.dram_tensor("ag2_in", [SHARD, CPAD], BF)
    ag2_out = nc.dram_tensor("ag2_out", [NP, CPAD], BF, addr_space="Shared")

    qctr = [0]

    def next_q():
        qctr[0] = (qctr[0] + 1) % 4
        return qctr[0]

    with tile.TileContext(nc) as tc:
        with (
            tc.tile_pool(name="const", bufs=1) as cpool,
            tc.tile_pool(name="p1", bufs=4) as p1pool,
            tc.tile_pool(name="meta1", bufs=6) as mpool,
            tc.tile_pool(name="gath", bufs=4) as gpool,
            tc.tile_pool(name="indp", bufs=3) as ipool,
            tc.tile_pool(name="mid", bufs=3) as midpool,
            tc.tile_pool(name="ps", bufs=2, space="PSUM") as pspool,
        ):
            # ---- constants ----
            iota_t = cpool.tile([128, 128], FP)
            nc.sync.dma_start(out=iota_t[:, :], in_=iota[:, :])
            b1_t = cpool.tile([HID, 1], FP)
            nc.sync.dma_start(out=b1_t[:, :], in_=b1col[:, :])
            w2_t = cpool.tile([HID, CPAD], BFIN)
            nc.sync.dma_start(out=w2_t[:, :], in_=w2p[:, :])
            b2_t = cpool.tile([128, CPAD], FP)
            nc.sync.dma_start(out=b2_t[:, :], in_=b2rep[:, :])
            w1k_t = cpool.tile([128, KT, HID], BFIN)
            for k in range(KT):
                nc.sync.dma_start(out=w1k_t[:, k, :], in_=w1[k * 128:(k + 1) * 128, :])

            # ---- phase 1: h1' = dinv .* (x @ W1) ----
            for t in range(SHARD // 128):
                psh = pspool.tile([128, HID], FP, space="PSUM", tag="ph1")
                for k in range(KT):
                    xt_t = p1pool.tile([128, 128], BFIN, tag="xt")
                    nc.sync.dma_start(
                        out=xt_t[:, :],
                        in_=xT[k * 128:(k + 1) * 128, t * 128:(t + 1) * 128])
                    nc.tensor.matmul(out=psh[:, :], lhsT=xt_t[:, :],
                                     rhs=w1k_t[:, k, :],
                                     start=(k == 0), stop=(k == KT - 1))
                dxt = p1pool.tile([128, 1], FP, tag="dx")
                nc.sync.dma_start(out=dxt[:, :], in_=dinv_x[t * 128:(t + 1) * 128, :])
                h1p = p1pool.tile([128, HID], BF, tag="h1p")
                nc.scalar.activation(out=h1p[:, :], in_=psh[:, :],
                                     func=mybir.ActivationFunctionType.Copy,
                                     scale=dxt[:, :1])
                nc.sync.dma_start(out=ag1_in[t * 128:(t + 1) * 128, :], in_=h1p[:, :])

            # ---- AG1 ----
            nc.gpsimd.collective_compute(
                "AllGather", mybir.AluOpType.bypass,
                replica_groups=[list(range(W))],
                ins=[ag1_in[:, :]], outs=[ag1_out[:, :]],
            )

            # ---- phase 3: L1 aggregation + relu + @W2 -> h2' ----
            off1 = np.zeros(NB + 1, dtype=np.int64)
            off1[1:] = np.cumsum(cpb1)
            for bb in range(NB):
                cpb = int(cpb1[bb])
                o0 = int(off1[bb])
                ixt = mpool.tile([128, MAXCPB1 * 8], mybir.dt.int16, tag="ix1")
                nc.sync.dma_start(out=ixt[:, :cpb * 8],
                                  in_=idx1[:, o0 * 8:(o0 + cpb) * 8])
                dlt = mpool.tile([128, MAXCPB1], FP, tag="dl1")
                nc.sync.dma_start(out=dlt[:, :cpb], in_=dl1[:, o0:o0 + cpb])
                drt = mpool.tile([128, 128], FP, tag="dr")
                nc.sync.dma_start(out=drt[:, :],
                                  in_=dinv_pr[:, bb * 128:(bb + 1) * 128])

                gbuf = gpool.tile([128, MAXCPB1, HID], BF, tag="g1")
                co = 0
                for q in range(Q):
                    kq = int(kq1[bb, q])
                    if kq == 0:
                        continue
                    nc.gpsimd.dma_gather(
                        gbuf[:, co:co + kq, :],
                        ag1_out[q * QS:(q + 1) * QS, :],
                        ixt[:, co * 8:(co + kq) * 8],
                        kq * 128, kq * 128, HID,
                        single_packet=False, queue_num=next_q(),
                    )
                    co += kq

                ind = ipool.tile([128, MAXCPB1, 128], BF, tag="i1")
                nc.vector.tensor_tensor(
                    out=ind[:, :cpb, :],
                    in0=dlt[:, :cpb].to_broadcast([128, cpb, 128]),
                    in1=iota_t[:, None, :].to_broadcast([128, cpb, 128]),
                    op=mybir.AluOpType.is_equal,
                )

                ps1 = pspool.tile([128, 128], FP, space="PSUM", tag="ps1")
                for ck in range(cpb):
                    nc.tensor.matmul(out=ps1[:, :], lhsT=gbuf[:, ck, :],
                                     rhs=ind[:, ck, :],
                                     start=(ck == 0), stop=(ck == cpb - 1))

                t1 = midpool.tile([128, 128], FP, tag="t1")
                nc.vector.tensor_tensor(out=t1[:, :], in0=ps1[:, :], in1=drt[:, :],
                                        op=mybir.AluOpType.mult)
                r1 = midpool.tile([128, 128], BF, tag="r1")
                nc.scalar.activation(out=r1[:, :], in_=t1[:, :],
                                     func=mybir.ActivationFunctionType.Relu,
                                     bias=b1_t[:, :1])
                ps2 = pspool.tile([128, CPAD], FP, space="PSUM", tag="ps2")
                nc.tensor.matmul(out=ps2[:, :], lhsT=r1[:, :], rhs=w2_t[:, :],
                                 start=True, stop=True)
                dpt = mpool.tile([128, 1], FP, tag="dp1")
                nc.sync.dma_start(out=dpt[:, :],
                                  in_=dinv_pc[bb * 128:(bb + 1) * 128, :])
                h2p = midpool.tile([128, CPAD], BF, tag="h2p")
                nc.scalar.activation(out=h2p[:, :], in_=ps2[:, :],
                                     func=mybir.ActivationFunctionType.Copy,
                                     scale=dpt[:, :1])
                nc.sync.dma_start(out=ag2_in[bb * 128:(bb + 1) * 128, :],
                                  in_=h2p[:, :])

            # ---- AG2 ----
            nc.gpsimd.collective_compute(
                "AllGather", mybir.AluOpType.bypass,
                replica_groups=[list(range(W))],
                ins=[ag2_in[:, :]], outs=[ag2_out[:, :]],
            )

            # ---- phase 5: L2 aggregation + b2 -> out ----
            off2 = np.zeros(NB + 1, dtype=np.int64)
            off2[1:] = np.cumsum(cpb2)
            for bb in range(NB):
                cpb = int(cpb2[bb])
                o0 = int(off2[bb])
                ixt = mpool.tile([128, MAXCPB2 * 8], mybir.dt.int16, tag="ix2")
                nc.sync.dma_start(out=ixt[:, :cpb * 8],
                                  in_=idx2[:, o0 * 8:(o0 + cpb) * 8])
                dlt = mpool.tile([128, MAXCPB2], FP, tag="dl2")
                nc.sync.dma_start(out=dlt[:, :cpb], in_=dl2[:, o0:o0 + cpb])

                gbuf = gpool.tile([128, MAXCPB2, CPAD], BF, tag="g2")
                co = 0
                for q in range(Q):
                    kq = int(kq2[bb, q])
                    if kq == 0:
                        continue
                    nc.gpsimd.dma_gather(
                        gbuf[:, co:co + kq, :],
                        ag2_out[q * QS:(q + 1) * QS, :],
                        ixt[:, co * 8:(co + kq) * 8],
                        kq * 128, kq * 128, CPAD,
                        single_packet=False, queue_num=next_q(),
                    )
                    co += kq

                ind = ipool.tile([128, MAXCPB2, 128], BF, tag="i2")
                nc.vector.tensor_tensor(
                    out=ind[:, :cpb, :],
                    in0=dlt[:, :cpb].to_broadcast([128, cpb, 128]),
                    in1=iota_t[:, None, :].to_broadcast([128, cpb, 128]),
                    op=mybir.AluOpType.is_equal,
                )

                ps3 = pspool.tile([128, CPAD], FP, space="PSUM", tag="ps3")
                for ck in range(cpb):
                    nc.tensor.matmul(out=ps3[:, :], lhsT=ind[:, ck, :],
                                     rhs=gbuf[:, ck, :],
                                     start=(ck == 0), stop=(ck == cpb - 1))

                dpt = mpool.tile([128, 1], FP, tag="dp2")
                nc.sync.dma_start(out=dpt[:, :],
                                  in_=dinv_pc[bb * 128:(bb + 1) * 128, :])
                t3 = midpool.tile([128, CPAD], FP, tag="t3")
                nc.scalar.activation(out=t3[:, :], in_=ps3[:, :],
                                     func=mybir.ActivationFunctionType.Copy,
                                     scale=dpt[:, :1])
                # self-loop diagonal: + dinv^2 * h2' (local ag2_in rows)
                h2b = mpool.tile([128, CPAD], BF, tag="h2b")
                nc.sync.dma_start(out=h2b[:, :],
                                  in_=ag2_in[bb * 128:(bb + 1) * 128, :])
                d2t = mpool.tile([128, 1], FP, tag="d2")
                nc.sync.dma_start(out=d2t[:, :],
                                  in_=dinv_pc[bb * 128:(bb + 1) * 128, :])
                sl = midpool.tile([128, CPAD], FP, tag="sl")
                nc.vector.tensor_scalar(sl[:, :], h2b[:, :], d2t[:, :1], None,
                                        mybir.AluOpType.mult)
                o3a = midpool.tile([128, CPAD], FP, tag="o3a")
                nc.vector.tensor_tensor(out=o3a[:, :], in0=t3[:, :], in1=sl[:, :],
                                        op=mybir.AluOpType.add)
                o3 = midpool.tile([128, CPAD], FP, tag="o3")
                nc.vector.tensor_tensor(out=o3[:, :], in0=o3a[:, :], in1=b2_t[:, :],
                                        op=mybir.AluOpType.add)
                nc.sync.dma_start(out=out_s[bb * 128:(bb + 1) * 128, :],
                                  in_=o3[:, :])

    nc.compile()
    return nc


# ======================================================================
# kernel() entry point
# ======================================================================
import os as _os


LAST_EXEC_NS = None
LAST_RES = None


def kernel(x, edge_index, W1, b1, W2, b2):
    """Full-input GCN kernel: shards across 8 NeuronCores internally."""
    global LAST_EXEC_NS, LAST_RES
    import numpy as _np

    trace = bool(int(_os.environ.get("GCN_TRACE", "0")))
    if trace:
        # Optional NTFF profiling shim (axon): non-fatal if unavailable.
        try:
            import sys as _sys
            import types as _types
            from trn_agent_boot.trn_boot import _ntff_profile_via_ctypes
            if "antenv.axon_hooks" not in _sys.modules:
                _hook = _ntff_profile_via_ctypes("/opt/axon/libaxon_pjrt.so")
                _m = _types.ModuleType("antenv.axon_hooks")
                _m.get_axon_ntff_profile_hook = lambda: _hook
                _m.set_axon_ntff_profile_hook = lambda h: None
                _sys.modules["antenv.axon_hooks"] = _m
        except Exception:
            trace = False

    from concourse.bass_utils import run_bass_kernel_spmd

    cfg = Cfg()
    per_core, meta, _ = preprocess(cfg, x, edge_index, W1, b1, W2, b2)
    nc = build(cfg, meta)
    res = run_bass_kernel_spmd(
        nc, per_core, core_ids=list(range(cfg.W)), trace=trace,
    )
    LAST_EXEC_NS = res.exec_time_ns
    LAST_RES = res
    outs = [res.results[c]["out_s"] for c in range(cfg.W)]
    return _np.ascontiguousarray(postprocess(cfg, outs, meta).astype(_np.float32))

